# revision 4
# baseline (speedup 1.0000x reference)
"""Trainium2 Bass kernel for nn_Attention_57380763075267 (sparse_attention).

Reference computation (B=4, N=2048, DIM=512, H=8, HD=64):
    qkv = (x @ qkv_w.T) -> q, v   (k == q)
    attn = softmax(mask(q @ q.T * HD**-0.5))
    out  = (attn @ v)  -> reshape -> @ proj_w.T + proj_b

Sharding: 8 cores = (batch b in 0..3) x (query-half ih in 0..1).
Each core computes the full attention for its 1024 query rows of batch b
(all 8 heads on-core, so the output projection needs no cross-core reduce).
No collectives: per-core inputs are pre-sliced on host, outputs re-assembled
on host.

Key implementation choices:
  * All score / P tiles live in the TRANSPOSED domain [j(keys) x i(queries)]
    so every matmul has a 512-wide f16 moving operand (1 cycle/row).
  * Key order is permuted per-core (queries first) so the same static
    program works on every core; permutation applied consistently to the
    mask rows and V rows on host (sum over j is order invariant).
  * Softmax without row-max subtraction (scores*SCALE is O(3), exp is safe
    in fp32) and without a separate row-sum pass: V is augmented with a
    ones column so Z arrives in the attention matmul output (row 64).
  * Mask applied post-exp as a fp16 multiply (keep = 1-mask), DVE 2x mode;
    a tunable subset of mask multiplies runs on GpSimd to offload the DVE.
  * Score matmuls for a head PAIR run concurrently on the PE via row
    tiling (K=64 at array rows 0-63 / 64-127); ONE 1024-wide exp per pair.
  * Engine balancing: v/q-phase PSUM->SBUF copies and the output bias adds
    run on ScalarE (in its exp-idle windows); 1/Z computed directly to f16.
"""

import numpy as np

import concourse.bacc as bacc
import concourse.tile as tile
from concourse import mybir
from concourse.bass_utils import run_bass_kernel_spmd

B, N, DIM, H = 4, 2048, 512, 8
HD = DIM // H          # 64
SCALE = HD ** -0.5     # 0.125
I = N // 2             # 1024 queries per core
NCORES = 8

F32 = mybir.dt.float32
F16 = mybir.dt.float16
EXP = mybir.ActivationFunctionType.Exp
QDT = F16

# Which mask multiplies run on GpSimd instead of DVE: (jt % GPS_MOD) in GPS_JT
# applies to the g=GPS_G half. GpSimd is ~3.4x slower per tile but otherwise
# idle; offloading trims the DVE critical path.
GPS_MOD = 4
GPS_JT = (1, 3)
GPS_G = (1,)


def build_nc(reps=1):
    """Build the per-core program. reps>1 wraps the body in a HW loop
    (used only for wall-clock benchmarking by repetition)."""
    import contextlib

    nc = bacc.Bacc(None)

    xTp = nc.declare_dram_parameter("xTp", [DIM, N], QDT, isOutput=False).ap()
    wT = nc.declare_dram_parameter("wT", [DIM, 2 * DIM], QDT, isOutput=False).ap()
    pwT = nc.declare_dram_parameter("pwT", [DIM, DIM], F16, isOutput=False).ap()
    pb = nc.declare_dram_parameter("pb", [DIM], F32, isOutput=False).ap()
    keepTp = nc.declare_dram_parameter("keepTp", [N, I], F16, isOutput=False).ap()
    outT = nc.declare_dram_parameter("outT", [DIM, I], F32, isOutput=True).ap()

    with tile.TileContext(nc) as tc:
        with (
            tc.tile_pool(name="singles", bufs=1) as singles,
            tc.tile_pool(name="pt", bufs=16) as pt_pool,
            tc.tile_pool(name="small", bufs=2) as small,
            tc.tile_pool(name="fin", bufs=2) as fin_pool,
            tc.tile_pool(name="ps", bufs=2, space="PSUM") as ps_pool,
            tc.tile_pool(name="po", bufs=4, space="PSUM") as po_pool,
            tc.For_i(0, reps, 1) if reps > 1 else contextlib.nullcontext(),
        ):
            # ---- resident SBUF tensors ----
            w_sb = singles.tile([128, 4, 2 * DIM], QDT)    # qkv_w.T  (c-chunk major)
            x_sb = singles.tile([128, 4, N], QDT)          # x[b].T   (c-chunk major)
            keep_sb = singles.tile([128, 16, I], F16)       # keep mask^T (j-tile major)
            q_sb = singles.tile([128, 4, N], QDT)          # q^T (head-pair major)
            v_sb = singles.tile([128, 16, H * (HD + 1)], F16)  # V': 8 x [64 v | 1] per j
            oh_sb = singles.tile([128, 4, I], F16)          # normalized O^T (hd-chunks)
            pw_sb = singles.tile([128, 4, DIM], F16)        # proj_w.T (hd-chunk major)
            pb_sb = singles.tile([128, 4], F32)             # proj bias (d-tile major)
            ones_sb = singles.tile([65, HD], F16)
            warm = singles.tile([1, 8], F32)

            # ---- phase-1-critical input DMAs, ordered so the first q tile and
            # the first mask tiles land as early as possible ----
            xv = xTp.rearrange("(t p) n -> p t n", p=128)
            wv = wT.rearrange("(t p) o -> p t o", p=128)
            kv = keepTp.rearrange("(t p) i -> p t i", p=128)
            nc.sync.dma_start(out=x_sb[:, :, 0:512], in_=xv[:, :, 0:512])
            nc.sync.dma_start(out=w_sb[:, :, 0:DIM], in_=wv[:, :, 0:DIM])
            nc.sync.dma_start(out=w_sb[:, :, DIM : 2 * DIM], in_=wv[:, :, DIM : 2 * DIM])
            nc.sync.dma_start(out=x_sb[:, :, 512:1024], in_=xv[:, :, 512:1024])
            nc.sync.dma_start(out=x_sb[:, :, 1024:1536], in_=xv[:, :, 1024:1536])
            nc.sync.dma_start(out=x_sb[:, :, 1536:2048], in_=xv[:, :, 1536:2048])
            # mask tiles ride the SWDGE queue in parallel with x/w on HWDGE
            for kq in range(4):
                nc.gpsimd.dma_start(
                    out=keep_sb[:, 4 * kq : 4 * kq + 4, :],
                    in_=kv[:, 4 * kq : 4 * kq + 4, :],
                )

            # warm the ACT exp table while DMAs run
            nc.gpsimd.memset(ones_sb, 1.0)
            nc.scalar.activation(out=warm, in_=ones_sb[0:1, 0:8], func=EXP, scale=1.0)

            # ones columns of V' (column 64 of every 65-wide head group)
            vview = v_sb.rearrange("p t (h e) -> p t h e", e=HD + 1)
            nc.gpsimd.memset(vview[:, :, :, HD : HD + 1], 1.0)

            # ---- q/v projection chunks (emitted interleaved with attention
            # so the per-engine FIFOs never stall long on a DMA). The psum
            # evacuation copies run on ScalarE for ot=0 / v (its exp-idle
            # window) and on DVE for the later q passes. ----
            def q_chunk(ot, nch, eng):
                psq = po_pool.tile([128, 512], F32, tag="po")
                for c4 in range(4):
                    nc.tensor.matmul(
                        psq,
                        lhsT=w_sb[:, c4, ot * 128 : (ot + 1) * 128],
                        rhs=x_sb[:, c4, nch * 512 : (nch + 1) * 512],
                        start=(c4 == 0),
                        stop=(c4 == 3),
                    )
                dst = q_sb[:, ot, nch * 512 : (nch + 1) * 512]
                if eng == "act":
                    nc.scalar.copy(dst, psq)
                else:
                    nc.vector.tensor_copy(dst, psq)

            def q_pass(ot):
                for nch in range(4):
                    q_chunk(ot, nch, "dve")

            def v_chunk(nt):
                psv = po_pool.tile([128, 512], F32, tag="po")
                for c4 in range(4):
                    nc.tensor.matmul(
                        psv,
                        lhsT=x_sb[:, c4, nt * 128 : (nt + 1) * 128],
                        rhs=w_sb[:, c4, DIM : 2 * DIM],
                        start=(c4 == 0),
                        stop=(c4 == 3),
                    )
                nc.scalar.copy(
                    vview[:, nt, :, 0:HD],
                    psv.rearrange("p (h e) -> p h e", e=HD),
                )

            # ---- phase 2 + 3: attention, then projection per query chunk.
            # The v-pass is interleaved into the first attention block (v tile
            # for key-tile jt is produced just before attn consumes it) so exp
            # work starts right after the first q tile instead of after a
            # serial phase 1. ----
            for ic in range(2):
                for hp in range(4):
                    if ic == 0 and hp == 1:
                        nc.sync.dma_start(
                            out=pw_sb, in_=pwT.rearrange("(t p) d -> p t d", p=128)
                        )
                        nc.sync.dma_start(
                            out=pb_sb, in_=pb.rearrange("(t p) -> p t", p=128)
                        )
                    if ic == 0 and hp > 0:
                        q_pass(hp)
                    po_e = po_pool.tile([128, 512], F32, tag="po")
                    po_o = po_pool.tile([128, 512], F32, tag="po")

                    def attn(pts, jt):
                        # O^T accumulation (row 64 collects Z via ones column)
                        nc.tensor.matmul(
                            po_e[0 : HD + 1, :],
                            lhsT=v_sb[:, jt, (2 * hp) * 65 : (2 * hp) * 65 + 65],
                            rhs=pts[0],
                            start=(jt == 0),
                            stop=(jt == 15),
                        )
                        nc.tensor.matmul(
                            po_o[0 : HD + 1, :],
                            lhsT=v_sb[:, jt, (2 * hp + 1) * 65 : (2 * hp + 1) * 65 + 65],
                            rhs=pts[1],
                            start=(jt == 0),
                            stop=(jt == 15),
                        )

                    pending = None  # attn deferred one tile: PE never stalls
                    for jt in range(16):
                        if ic == 0 and hp == 0 and jt % 4 == 0:
                            q_chunk(0, jt // 4, "act")
                        kp = keep_sb[:, jt, ic * 512 : (ic + 1) * 512]
                        ps2 = ps_pool.tile([128, 1024], F32, tag="ps")
                        for g in range(2):
                            nc.tensor.matmul(
                                ps2[:, 512 * g : 512 * g + 512],
                                lhsT=q_sb[64 * g : 64 * g + 64, hp,
                                          jt * 128 : (jt + 1) * 128],
                                rhs=q_sb[64 * g : 64 * g + 64, hp,
                                         ic * 512 : (ic + 1) * 512],
                                start=True,
                                stop=True,
                            )
                        ptw = pt_pool.tile([128, 1024], F16, tag="pt")
                        pts = [ptw[:, 0:512], ptw[:, 512:1024]]
                        # one 1024-wide exp for the head pair
                        nc.scalar.activation(
                            out=ptw, in_=ps2, func=EXP, scale=float(SCALE)
                        )
                        for g in range(2):
                            if (jt % GPS_MOD) in GPS_JT and g in GPS_G:
                                nc.gpsimd.tensor_mul(pts[g], pts[g], kp)
                            else:
                                nc.vector.tensor_mul(pts[g], pts[g], kp)
                        if ic == 0 and hp == 0:
                            v_chunk(jt)
                        if pending is not None:
                            attn(*pending)
                        pending = (pts, jt)
                    attn(*pending)
                    # normalization: oh = O^T * (1/Z); 1/Z computed straight to
                    # f16, then broadcast along partitions via K=1 matmuls
                    # (both heads share one PSUM bank).
                    rzh = small.tile([65, 1024], F16, tag="rzh")
                    with nc.allow_low_precision(reason="1/Z consumed as f16 matmul rhs"):
                        nc.vector.reciprocal(rzh[64:65, 0:512], po_e[HD : HD + 1, :])
                        nc.vector.reciprocal(rzh[64:65, 512:1024], po_o[HD : HD + 1, :])
                    przp = po_pool.tile([128, 512], F32, tag="po")
                    nc.tensor.matmul(
                        przp[0:64, :],
                        lhsT=ones_sb[64:65, :],
                        rhs=rzh[64:65, 0:512],
                        start=True,
                        stop=True,
                    )
                    nc.tensor.matmul(
                        przp[64:128, :],
                        lhsT=ones_sb[64:65, :],
                        rhs=rzh[64:65, 512:1024],
                        start=True,
                        stop=True,
                    )
                    rzr = small.tile([128, 512], F32, tag="rzr")
                    nc.vector.tensor_copy(rzr, przp)
                    nc.vector.tensor_mul(
                        oh_sb[0:64, hp, ic * 512 : (ic + 1) * 512],
                        po_e[0:HD, :],
                        rzr[0:64, :],
                    )
                    nc.vector.tensor_mul(
                        oh_sb[64:128, hp, ic * 512 : (ic + 1) * 512],
                        po_o[0:HD, :],
                        rzr[64:128, :],
                    )

                # projection for this query chunk (overlaps next chunk's
                # attention); bias add on ScalarE
                for dt4 in range(4):
                    pf = po_pool.tile([128, 512], F32, tag="po")
                    for hp in range(4):
                        nc.tensor.matmul(
                            pf,
                            lhsT=pw_sb[:, hp, dt4 * 128 : (dt4 + 1) * 128],
                            rhs=oh_sb[:, hp, ic * 512 : (ic + 1) * 512],
                            start=(hp == 0),
                            stop=(hp == 3),
                        )
                    fin = fin_pool.tile([128, 512], F32, tag="fin")
                    nc.scalar.add(fin, pf, pb_sb[:, dt4 : dt4 + 1])
                    nc.sync.dma_start(
                        out=outT[dt4 * 128 : (dt4 + 1) * 128, ic * 512 : (ic + 1) * 512],
                        in_=fin,
                    )

    nc.compile()
    return nc


def prep_inputs(x, qkv_w, proj_w, proj_b, freq_attn_mask):
    """Build the 8 per-core input maps (host-side slicing/permutation)."""
    x = np.asarray(x, dtype=np.float32)
    qkv_w = np.asarray(qkv_w, dtype=np.float32)
    proj_w = np.asarray(proj_w, dtype=np.float32)
    proj_b = np.asarray(proj_b, dtype=np.float32)
    mask = np.asarray(freq_attn_mask)

    qdt = np.float16 if QDT == F16 else np.float32
    wT = np.ascontiguousarray(qkv_w.T).astype(qdt)           # [512, 1024]
    pwT = np.ascontiguousarray(proj_w.T).astype(np.float16)  # [512, 512] f16
    keepT = np.ascontiguousarray((1 - mask).T.astype(np.float16))  # [2048 j, 2048 i]

    in_maps = []
    for c in range(NCORES):
        b, ih = c // 2, c % 2
        lo, hi = ih * I, (ih + 1) * I
        perm = np.r_[lo:hi, 0:lo, hi:N]  # queries first, rest after
        xT = x[b].T  # [512, 2048]
        in_maps.append(
            {
                "xTp": np.ascontiguousarray(xT[:, perm]).astype(qdt),
                "wT": wT,
                "pwT": pwT,
                "pb": proj_b,
                "keepTp": np.ascontiguousarray(keepT[perm][:, lo:hi]),
            }
        )
    return in_maps


def assemble(results):
    out = np.empty((B, N, DIM), dtype=np.float32)
    for c in range(NCORES):
        b, ih = c // 2, c % 2
        out[b, ih * I : (ih + 1) * I, :] = results[c]["outT"].T
    return out


_NC_CACHE = None


def kernel(x, qkv_w, proj_w, proj_b, freq_attn_mask):
    global _NC_CACHE
    if _NC_CACHE is None:
        _NC_CACHE = build_nc()
    nc = _NC_CACHE
    in_maps = prep_inputs(x, qkv_w, proj_w, proj_b, freq_attn_mask)
    res = run_bass_kernel_spmd(nc, in_maps, list(range(NCORES)))
    return assemble(res.results)
